# revision 6
# baseline (speedup 1.0000x reference)
"""Trainium2 Bass kernel for nn_CombineNode_7395933684091 (gnn_message_passing).

Hierarchy: 128 leaf terms (each D=1024 -> H=32), 16 internal terms
(concat of 8 children hiddens, 256 -> 32), 1 root (concat of 16
internal hiddens, 512 -> 32); every term also has a 1-dim predict head.
All matmuls followed by tanh.

Strategy: data-parallel over batch across 8 cores (Bc = 1024 rows per
core), weights replicated. On-chip layout keeps hidden features on the
PARTITION axis ("h^T layout": tiles are [features, batch]), so every
level's contraction is a natural PE matmul and the child-concat is just
stacking partition tiles. All matmul operands are bf16 (full-rate PE,
FWL halves LDWEIGHTS, half the DMA bytes of f32); PSUM stays f32.

The 64 (panel, batch-half, group) leaf tiles are emitted as a software-
pipelined slot stream: each slot emits its 8 accumulating leaf matmuls,
then the *previous* slot's tanh + combine matmul, so the PE never
head-of-line blocks on the ACT engine. Per-term predict heads ride as
block-diagonal columns in the combine/root stationaries; their raw dots
are copied off PSUM by the (otherwise idle) DVE and DMA'd out
unactivated — bias + tanh for the 145 predict rows happen on the host.

DMA: inputs split across the sync (xt), vector (panel-0 weights),
gpsimd (panel-1..3 weights) engine queues so issue doesn't serialize;
first chunks are quartered so the first matmul can start right after
the framework preamble.
"""

import numpy as np

B, D, H = 8192, 1024, 32
L, I, CPI = 128, 16, 8
NCORES = 8
BC = B // NCORES      # 1024 batch rows per core
BN = 512              # batch tile width (one PSUM bank of f32)
NBH = BC // BN        # 2 batch halves
KC = D // 128         # 8 contraction chunks for the leaf level
NPANEL = 4            # leaf panels (8 groups of 4 leaves each)
GPP = 8               # groups per panel
NOUT = L + I + 1      # 145

MM_DT = "bfloat16"

_CACHE = {}


def _build_nc():
    from contextlib import ExitStack

    import concourse.mybir as mybir
    import concourse.tile as tile
    from concourse import bacc

    f32 = mybir.dt.float32
    Tanh = mybir.ActivationFunctionType.Tanh
    mmdt = getattr(mybir.dt, MM_DT)

    nc = bacc.Bacc("TRN2", target_bir_lowering=False, debug=False)

    xt = nc.dram_tensor("xt", [D, BC], mmdt, kind="ExternalInput")
    lw = nc.dram_tensor("lw", [D, L * H], mmdt, kind="ExternalInput")
    lb = nc.dram_tensor("lb", [128, 32], f32, kind="ExternalInput")
    # fused internal-trans + leaf-predict stationary: per (node i, chunk j)
    # a [128, 64] block: cols 0:32 int_W chunk, cols 32+4j+c leaf Wp diag
    cw = nc.dram_tensor("cw", [128, I * 2 * 64], mmdt, kind="ExternalInput")
    intb = nc.dram_tensor("intb", [128, 4], f32, kind="ExternalInput")
    # fused root-trans + int-predict stationary: per panel q a [128, 48]
    # block: cols 0:32 root_W chunk, cols 32:48 int Wp diag
    rw2 = nc.dram_tensor("rw2", [128, NPANEL * 48], mmdt, kind="ExternalInput")
    rootb = nc.dram_tensor("rootb", [32, 1], f32, kind="ExternalInput")
    rootwp = nc.dram_tensor("rootwp", [32, 1], mmdt, kind="ExternalInput")
    # raw (pre-bias, pre-tanh) predict dots; host applies bias + tanh
    out = nc.dram_tensor("out", [NOUT, BC], f32, kind="ExternalOutput")

    mm = nc.tensor.matmul

    with tile.TileContext(nc) as tc, ExitStack() as ctx:
        consts = ctx.enter_context(tc.tile_pool(name="consts", bufs=1))
        wpool = ctx.enter_context(tc.tile_pool(name="wpool", bufs=1))
        work = ctx.enter_context(tc.tile_pool(name="work", bufs=4))
        keep = ctx.enter_context(tc.tile_pool(name="keep", bufs=1))
        psum = ctx.enter_context(tc.tile_pool(name="psum", bufs=1, space="PSUM"))

        # ---- PE pre-warm: input DMA cannot issue before the framework
        # preamble ends (~7us), so ~4us of dummy f32 matmuls bridges the
        # first-chunk transfer latency and trips the HAM clock un-throttle
        # right as real data lands.
        warm_l = nc.const_aps.tensor(0.0, (128, 48), f32)
        warm_r = nc.const_aps.tensor(0.0, (128, 64), f32)
        pwarm = psum.tile([48, BN], f32, tag="prc", bufs=1, name="pwarm")
        for _ in range(22):
            mm(pwarm[:, 0:64], warm_l, warm_r, start=True, stop=True,
               skip_group_check=True)

        # ---- input DMAs, split across engine queues ------------------------
        # sync: xt left halves (bn0, early chunks quartered), right halves,
        # then panel 2-3 weights
        xt_sb = consts.tile([128, KC * BC], mmdt, name="xt_sb")
        for k in range(KC):
            base = k * BC
            if k < 4:
                nc.sync.dma_start(
                    xt_sb[:, base:base + 256], xt[k * 128:(k + 1) * 128, 0:256]
                )
                nc.sync.dma_start(
                    xt_sb[:, base + 256:base + 512],
                    xt[k * 128:(k + 1) * 128, 256:512],
                )
            else:
                nc.sync.dma_start(
                    xt_sb[:, base:base + BN], xt[k * 128:(k + 1) * 128, 0:BN]
                )
        for k in range(KC):
            nc.sync.dma_start(
                xt_sb[:, k * BC + BN:(k + 1) * BC],
                xt[k * 128:(k + 1) * 128, BN:BC],
            )

        # gpsimd: panel-0 weights (early chunks quartered), then panel 1
        wps = {}
        wps[0] = wpool.tile([128, KC * 1024], mmdt, tag="wp0", name="wp0")
        for k in range(KC):
            base = k * 1024
            if k < 4:
                nc.gpsimd.dma_start(
                    wps[0][:, base:base + 256], lw[k * 128:(k + 1) * 128, 0:256]
                )
                nc.gpsimd.dma_start(
                    wps[0][:, base + 256:base + 512],
                    lw[k * 128:(k + 1) * 128, 256:512],
                )
            else:
                nc.gpsimd.dma_start(
                    wps[0][:, base:base + 512], lw[k * 128:(k + 1) * 128, 0:512]
                )
        for k in range(KC):
            nc.gpsimd.dma_start(
                wps[0][:, k * 1024 + 512:(k + 1) * 1024],
                lw[k * 128:(k + 1) * 128, 512:1024],
            )

        # scalar: small constants (then it only runs ACTs)
        lb_sb = consts.tile([128, 32], f32, name="lb_sb")
        nc.scalar.dma_start(lb_sb[:], lb[:])
        intb_sb = consts.tile([128, 4], f32, name="intb_sb")
        nc.scalar.dma_start(intb_sb[:], intb[:])
        cw_sb = consts.tile([128, I * 2 * 64], mmdt, name="cw_sb")
        nc.scalar.dma_start(cw_sb[:], cw[:])
        rw2_sb = consts.tile([128, NPANEL * 48], mmdt, name="rw2_sb")
        nc.scalar.dma_start(rw2_sb[:], rw2[:])
        rootb_sb = consts.tile([32, 1], f32, name="rootb_sb")
        nc.scalar.dma_start(rootb_sb[:], rootb[:])
        rootwp_sb = consts.tile([32, 1], mmdt, name="rootwp_sb")
        nc.scalar.dma_start(rootwp_sb[:], rootwp[:])

        # panel 1 on gpsimd (after wp0), panels 2-3 on sync (after xt)
        for q, eng in ((1, nc.gpsimd), (2, nc.sync), (3, nc.sync)):
            wps[q] = wpool.tile([128, KC * 1024], mmdt, tag=f"wp{q}", name=f"wp{q}")
            for k in range(KC):
                eng.dma_start(
                    wps[q][:, k * 1024:(k + 1) * 1024],
                    lw[k * 128:(k + 1) * 128, q * 1024:(q + 1) * 1024],
                )

        inth = {}   # (panel, bn) -> [128, BN] tile: int nodes 4p..4p+3 h^T
        pcombs = {}  # il -> in-flight combine PSUM tile

        slots = [(p, bn, gl) for p in range(NPANEL) for bn in range(NBH)
                 for gl in range(GPP)]

        def leaf_mms(p, bn, gl):
            wp = wps[p]
            pg = psum.tile([128, BN], f32, tag="pg", bufs=4, name=f"pg{p}{bn}{gl}")
            for k in range(KC):
                mm(
                    pg[:],
                    wp[:, k * 1024 + gl * 128:k * 1024 + (gl + 1) * 128],
                    xt_sb[:, k * BC + bn * BN:k * BC + bn * BN + BN],
                    start=(k == 0),
                    stop=(k == KC - 1),
                )
            return pg

        def deferred(p, bn, gl, pg):
            ith = inth[(p, bn)]
            lh = work.tile([128, BN], mmdt, tag="lh", name=f"lh{p}{bn}{gl}")
            nc.scalar.activation(
                lh[:], pg[:], Tanh, bias=lb_sb[:, GPP * p + gl:GPP * p + gl + 1]
            )
            il, j = divmod(gl, 2)
            i = 4 * p + il
            if j == 0:
                pcombs[il] = psum.tile([64, BN], f32, tag="pcomb", bufs=2,
                                       name=f"pc{p}{bn}{il}")
            pc = pcombs[il]
            mm(
                pc[:],
                cw_sb[:, (2 * i + j) * 64:(2 * i + j + 1) * 64],
                lh[:],
                start=(j == 0),
                stop=(j == 1),
                skip_group_check=True,
            )
            if j == 1:
                pcombs.pop(il)
                nc.scalar.activation(
                    ith[32 * il:32 * il + 32, :],
                    pc[0:32, :],
                    Tanh,
                    bias=intb_sb[32 * il:32 * il + 32, p:p + 1],
                )
                lp = work.tile([8, BN], f32, tag="lp", bufs=8, name=f"lp{i}{bn}")
                nc.vector.tensor_copy(lp[:], pc[32:40, :])
                nc.gpsimd.dma_start(
                    out[8 * i:8 * i + 8, bn * BN:bn * BN + BN], lp[:]
                )

        def root_mm(bn, q, prc):
            mm(
                prc[:],
                rw2_sb[:, 48 * q:48 * (q + 1)],
                inth[(q, bn)][:],
                start=(q == 0),
                stop=(q == NPANEL - 1),
                skip_group_check=True,
            )

        def root_finish(bn, prc):
            rh = work.tile([32, BN], mmdt, tag="rh", bufs=2, name=f"rh{bn}")
            nc.scalar.activation(rh[:], prc[0:32, :], Tanh, bias=rootb_sb[:, 0:1])
            prp = psum.tile([1, BN], f32, tag="prp", bufs=1, name=f"prp{bn}")
            mm(prp[:], rootwp_sb[:], rh[:], start=True, stop=True)
            ip = work.tile([16, BN], f32, tag="ip", bufs=2, name=f"ip{bn}")
            nc.vector.tensor_copy(ip[:], prc[32:48, :])
            rp = work.tile([1, BN], f32, tag="rp", bufs=2, name=f"rp{bn}")
            nc.vector.tensor_copy(rp[:], prp[:])
            nc.sync.dma_start(out[L:L + I, bn * BN:bn * BN + BN], ip[:])
            nc.sync.dma_start(out[L + I:NOUT, bn * BN:bn * BN + BN], rp[:])

        # ---- head: k-outer wave over groups 0-2 of (panel 0, bn 0) so the
        # PE consumes each arriving (xt, wp0) chunk as soon as it lands
        inth[(0, 0)] = keep.tile([128, BN], mmdt, tag="inth00", name="inth00")
        wave = []
        for g in range(3):
            wave.append(psum.tile([128, BN], f32, tag="pg", bufs=4,
                                  name=f"pgw{g}"))
        for k in range(KC):
            for g in range(3):
                mm(
                    wave[g][:],
                    wps[0][:, k * 1024 + g * 128:k * 1024 + (g + 1) * 128],
                    xt_sb[:, k * BC:k * BC + BN],
                    start=(k == 0),
                    stop=(k == KC - 1),
                )
        pgs = {0: wave[0], 1: wave[1], 2: wave[2]}

        # ---- pipelined slot stream ----------------------------------------
        prcs = {}
        for t, (p, bn, gl) in enumerate(slots):
            if (p, bn) not in inth:
                inth[(p, bn)] = keep.tile([128, BN], mmdt, tag=f"inth{p}{bn}",
                                          name=f"inth{p}{bn}")
            if t >= 3:
                pgs[t] = leaf_mms(p, bn, gl)
            # drain deferred work: two at slots 3/4 (wave warm-up), then one
            if t == 3:
                todo = [0, 1]
            elif t == 4:
                todo = [2, 3]
            elif t >= 5:
                todo = [t - 1]
            else:
                todo = []
            for s in todo:
                sp, sbn, sgl = slots[s]
                deferred(sp, sbn, sgl, pgs.pop(s))

            # root for bn0 right after its last combine lands (slot 57);
            # spread bn1's first three root matmuls into later slots
            if (p, bn, gl) == (3, 1, 0):
                prcs[0] = psum.tile([48, BN], f32, tag="prc", bufs=1, name="prc0")
                for q in range(NPANEL):
                    root_mm(0, q, prcs[0])
                root_finish(0, prcs[0])
            elif (p, bn, gl) == (3, 1, 4):
                prcs[1] = psum.tile([48, BN], f32, tag="prc", bufs=1, name="prc1")
                root_mm(1, 0, prcs[1])
            elif (p, bn, gl) in ((3, 1, 5), (3, 1, 6)):
                root_mm(1, gl - 4, prcs[1])

        sp, sbn, sgl = slots[63]
        deferred(sp, sbn, sgl, pgs.pop(63))
        root_mm(1, 3, prcs[1])
        root_finish(1, prcs[1])

    nc.compile()
    return nc


def _pack_weights(inp):
    f = np.float32
    int_W = np.asarray(inp["int_W"], f)
    int_b = np.asarray(inp["int_b"], f)
    root_W = np.asarray(inp["root_W"], f)
    root_b = np.asarray(inp["root_b"], f)
    leaf_Wp = np.asarray(inp["leaf_Wp"], f)
    int_Wp = np.asarray(inp["int_Wp"], f)
    root_Wp = np.asarray(inp["root_Wp"], f)

    w = {}
    w["lw"] = np.ascontiguousarray(
        np.asarray(inp["leaf_W"], f).transpose(1, 0, 2).reshape(D, L * H)
    )
    w["lb"] = np.ascontiguousarray(np.asarray(inp["leaf_b"], f).reshape(32, 128).T)

    cw = np.zeros((128, I * 2 * 64), f)
    for i in range(I):
        for j in range(2):
            base = (2 * i + j) * 64
            # int_W chunk j of node i: rows (c*32+h) = child (4j+c) hidden h
            cw[:, base:base + 32] = int_W[i, 128 * j:128 * (j + 1), :]
            for c in range(4):
                lv = 8 * i + 4 * j + c
                cw[c * 32:(c + 1) * 32, base + 32 + 4 * j + c] = leaf_Wp[lv, :, 0]
    w["cw"] = cw
    w["intb"] = np.ascontiguousarray(int_b.reshape(4, 128).T)

    rw2 = np.zeros((128, NPANEL * 48), f)
    for q in range(NPANEL):
        rw2[:, 48 * q:48 * q + 32] = root_W[128 * q:128 * (q + 1), :]
        for c in range(4):
            iv = 4 * q + c
            rw2[c * 32:(c + 1) * 32, 48 * q + 32 + 4 * q + c] = int_Wp[iv, :, 0]
    w["rw2"] = rw2
    w["rootb"] = np.ascontiguousarray(root_b.reshape(32, 1))
    w["rootwp"] = np.ascontiguousarray(root_Wp.reshape(32, 1))

    import ml_dtypes

    bf16 = ml_dtypes.bfloat16
    for k in ("lw", "cw", "rw2", "rootwp"):
        w[k] = np.ascontiguousarray(w[k].astype(bf16))
    return w


def kernel(**inputs):
    import ml_dtypes

    from concourse.bass_utils import run_bass_kernel_spmd

    nc = _CACHE.get("nc")
    if nc is None:
        nc = _CACHE["nc"] = _build_nc()

    x = np.asarray(inputs["x"], np.float32)
    w = _pack_weights(inputs)
    in_maps = []
    for c in range(NCORES):
        m = dict(w)
        m["xt"] = np.ascontiguousarray(
            x[c * BC:(c + 1) * BC, :].T.astype(ml_dtypes.bfloat16)
        )
        in_maps.append(m)

    res = run_bass_kernel_spmd(nc, in_maps, core_ids=list(range(NCORES)))
    _CACHE["last_res"] = res
    outs = [np.asarray(res.results[c]["out"], np.float32) for c in range(NCORES)]
    raw = np.concatenate(outs, axis=1)  # [145, B] raw predict dots
    bias = np.concatenate([
        np.asarray(inputs["leaf_bp"], np.float32)[:, 0],
        np.asarray(inputs["int_bp"], np.float32)[:, 0],
        np.asarray(inputs["root_bp"], np.float32),
    ])
    full = np.tanh(raw + bias[:, None])[:, :, None]
    return full.astype(np.float32)


# revision 11
# speedup vs baseline: 1.0106x; 1.0106x over previous
"""Trainium2 Bass kernel for nn_CombineNode_7395933684091 (gnn_message_passing).

Hierarchy: 128 leaf terms (each D=1024 -> H=32), 16 internal terms
(concat of 8 children hiddens, 256 -> 32), 1 root (concat of 16
internal hiddens, 512 -> 32); every term also has a 1-dim predict head.
All matmuls followed by tanh.

Strategy: data-parallel over batch across 8 cores (Bc = 1024 rows per
core), weights replicated. On-chip layout keeps hidden features on the
PARTITION axis ("h^T layout": tiles are [features, batch]), so every
level's contraction is a natural PE matmul and the child-concat is just
stacking partition tiles. All matmul operands are bf16 (full-rate PE,
FWL halves LDWEIGHTS, half the DMA bytes of f32); PSUM stays f32.

The 64 (panel, batch-half, group) leaf tiles are emitted as a software-
pipelined slot stream: each slot emits its 8 accumulating leaf matmuls,
then the *previous* slot's tanh + combine matmul, so the PE never
head-of-line blocks on the ACT engine. Per-term predict heads ride as
block-diagonal columns in the combine/root stationaries; their raw dots
are copied off PSUM by the (otherwise idle) DVE and DMA'd out
unactivated — bias + tanh for the 145 predict rows happen on the host.

DMA: inputs split across the sync (xt), vector (panel-0 weights),
gpsimd (panel-1..3 weights) engine queues so issue doesn't serialize;
first chunks are quartered so the first matmul can start right after
the framework preamble.
"""

import numpy as np

B, D, H = 8192, 1024, 32
L, I, CPI = 128, 16, 8
NCORES = 8
BC = B // NCORES      # 1024 batch rows per core
BN = 512              # batch tile width (one PSUM bank of f32)
NBH = BC // BN        # 2 batch halves
KC = D // 128         # 8 contraction chunks for the leaf level
NPANEL = 4            # leaf panels (8 groups of 4 leaves each)
GPP = 8               # groups per panel
NOUT = L + I + 1      # 145

MM_DT = "bfloat16"

_CACHE = {}


def _build_nc():
    from contextlib import ExitStack

    import concourse.mybir as mybir
    import concourse.tile as tile
    from concourse import bacc

    f32 = mybir.dt.float32
    Tanh = mybir.ActivationFunctionType.Tanh
    mmdt = getattr(mybir.dt, MM_DT)

    nc = bacc.Bacc("TRN2", target_bir_lowering=False, debug=False)

    xt = nc.dram_tensor("xt", [D, BC], mmdt, kind="ExternalInput")
    lw = nc.dram_tensor("lw", [D, L * H], mmdt, kind="ExternalInput")
    lb = nc.dram_tensor("lb", [128, 32], f32, kind="ExternalInput")
    # fused internal-trans + leaf-predict stationary: per (node i, chunk j)
    # a [128, 64] block: cols 0:32 int_W chunk, cols 32+4j+c leaf Wp diag
    cw = nc.dram_tensor("cw", [128, I * 2 * 64], mmdt, kind="ExternalInput")
    intb = nc.dram_tensor("intb", [128, 4], f32, kind="ExternalInput")
    # fused root-trans + int-predict stationary: per panel q a [128, 48]
    # block: cols 0:32 root_W chunk, cols 32:48 int Wp diag
    rw2 = nc.dram_tensor("rw2", [128, NPANEL * 48], mmdt, kind="ExternalInput")
    rootb = nc.dram_tensor("rootb", [32, 1], f32, kind="ExternalInput")
    rootwp = nc.dram_tensor("rootwp", [32, 1], mmdt, kind="ExternalInput")
    # raw (pre-bias, pre-tanh) predict dots; host applies bias + tanh
    out = nc.dram_tensor("out", [NOUT, BC], f32, kind="ExternalOutput")

    mm = nc.tensor.matmul

    with tile.TileContext(nc) as tc, ExitStack() as ctx:
        consts = ctx.enter_context(tc.tile_pool(name="consts", bufs=1))
        wpool = ctx.enter_context(tc.tile_pool(name="wpool", bufs=1))
        work = ctx.enter_context(tc.tile_pool(name="work", bufs=4))
        keep = ctx.enter_context(tc.tile_pool(name="keep", bufs=1))
        psum = ctx.enter_context(tc.tile_pool(name="psum", bufs=1, space="PSUM"))

        # ---- PE pre-warm: input DMA cannot issue before the framework
        # preamble ends (~7us), so ~4us of dummy f32 matmuls bridges the
        # first-chunk transfer latency and trips the HAM clock un-throttle
        # right as real data lands.
        warm_l = nc.const_aps.tensor(0.0, (128, 48), f32)
        warm_r = nc.const_aps.tensor(0.0, (128, 64), f32)
        pwarm = psum.tile([48, BN], f32, tag="prc", bufs=2, name="pwarm")
        for _ in range(10):
            mm(pwarm[:, 0:64], warm_l, warm_r, start=True, stop=True,
               skip_group_check=True)
        # dummy reader so the BIR verifier keeps the warm-up writes
        wjunk = work.tile([1, 8], f32, tag="wjunk", bufs=1, name="wjunk")
        nc.vector.tensor_copy(wjunk[:], pwarm[0:1, 0:8])

        # ---- input DMAs, split across engine queues ------------------------
        # sync: xt left halves (bn0, early chunks quartered), right halves,
        # then panel 2-3 weights
        xt_sb = consts.tile([128, KC * BC], mmdt, name="xt_sb")
        for k in range(KC):
            nc.sync.dma_start(
                xt_sb[:, k * BC:k * BC + BN], xt[k * 128:(k + 1) * 128, 0:BN]
            )
        for k in range(KC):
            nc.sync.dma_start(
                xt_sb[:, k * BC + BN:(k + 1) * BC],
                xt[k * 128:(k + 1) * 128, BN:BC],
            )

        # gpsimd: panel-0 weights (early chunks quartered), then panel 1
        wps = {}
        wps[0] = wpool.tile([128, KC * 1024], mmdt, tag="wp0", name="wp0")
        for k in range(KC):
            nc.gpsimd.dma_start(
                wps[0][:, k * 1024:k * 1024 + 512], lw[k * 128:(k + 1) * 128, 0:512]
            )
        for k in range(KC):
            nc.gpsimd.dma_start(
                wps[0][:, k * 1024 + 512:(k + 1) * 1024],
                lw[k * 128:(k + 1) * 128, 512:1024],
            )

        # scalar: small constants (then it only runs ACTs)
        lb_sb = consts.tile([128, 32], f32, name="lb_sb")
        nc.scalar.dma_start(lb_sb[:], lb[:])
        intb_sb = consts.tile([128, 4], f32, name="intb_sb")
        nc.scalar.dma_start(intb_sb[:], intb[:])
        cw_sb = consts.tile([128, I * 2 * 64], mmdt, name="cw_sb")
        nc.scalar.dma_start(cw_sb[:], cw[:])
        rw2_sb = consts.tile([128, NPANEL * 48], mmdt, name="rw2_sb")
        nc.scalar.dma_start(rw2_sb[:], rw2[:])
        rootb_sb = consts.tile([32, 1], f32, name="rootb_sb")
        nc.scalar.dma_start(rootb_sb[:], rootb[:])
        rootwp_sb = consts.tile([32, 1], mmdt, name="rootwp_sb")
        nc.scalar.dma_start(rootwp_sb[:], rootwp[:])

        # panel 1 on gpsimd (after wp0), panels 2-3 on sync (after xt)
        for q, eng in ((1, nc.gpsimd), (2, nc.sync), (3, nc.sync)):
            wps[q] = wpool.tile([128, KC * 1024], mmdt, tag=f"wp{q}", name=f"wp{q}")
            for k in range(KC):
                eng.dma_start(
                    wps[q][:, k * 1024:(k + 1) * 1024],
                    lw[k * 128:(k + 1) * 128, q * 1024:(q + 1) * 1024],
                )

        inth = {}   # (panel, bn) -> [128, BN] tile: int nodes 4p..4p+3 h^T
        pcombs = {}  # il -> in-flight combine PSUM tile

        slots = [(p, bn, gl) for p in range(NPANEL) for bn in range(NBH)
                 for gl in range(GPP)]

        def leaf_mms(p, bn, gl):
            wp = wps[p]
            pg = psum.tile([128, BN], f32, tag="pg", bufs=3, name=f"pg{p}{bn}{gl}")
            for k in range(KC):
                mm(
                    pg[:],
                    wp[:, k * 1024 + gl * 128:k * 1024 + (gl + 1) * 128],
                    xt_sb[:, k * BC + bn * BN:k * BC + bn * BN + BN],
                    start=(k == 0),
                    stop=(k == KC - 1),
                )
            return pg

        def deferred(p, bn, gl, pg):
            ith = inth[(p, bn)]
            lh = work.tile([128, BN], mmdt, tag="lh", name=f"lh{p}{bn}{gl}")
            nc.scalar.activation(
                lh[:], pg[:], Tanh, bias=lb_sb[:, GPP * p + gl:GPP * p + gl + 1]
            )
            il, j = divmod(gl, 2)
            i = 4 * p + il
            if j == 0:
                pcombs[il] = psum.tile([64, BN], f32, tag="pcomb", bufs=2,
                                       name=f"pc{p}{bn}{il}")
            pc = pcombs[il]
            mm(
                pc[:],
                cw_sb[:, (2 * i + j) * 64:(2 * i + j + 1) * 64],
                lh[:],
                start=(j == 0),
                stop=(j == 1),
                skip_group_check=True,
            )
            if j == 1:
                pcombs.pop(il)
                nc.scalar.activation(
                    ith[32 * il:32 * il + 32, :],
                    pc[0:32, :],
                    Tanh,
                    bias=intb_sb[32 * il:32 * il + 32, p:p + 1],
                )
                lp = work.tile([8, BN], f32, tag="lp", bufs=8, name=f"lp{i}{bn}")
                nc.vector.tensor_copy(lp[:], pc[32:40, :])
                eng = nc.sync if p == 3 else nc.gpsimd
                eng.dma_start(
                    out[8 * i:8 * i + 8, bn * BN:bn * BN + BN], lp[:]
                )

        def root_mm(bn, q, prc):
            mm(
                prc[:],
                rw2_sb[:, 48 * q:48 * (q + 1)],
                inth[(q, bn)][:],
                start=(q == 0),
                stop=(q == NPANEL - 1),
                skip_group_check=True,
            )

        # ---- head: k-outer wave over groups 0-2 of (panel 0, bn 0) so the
        # PE consumes each arriving (xt, wp0) chunk as soon as it lands
        inth[(0, 0)] = keep.tile([128, BN], mmdt, tag="inth00", name="inth00")
        wave = []
        for g in range(3):
            wave.append(psum.tile([128, BN], f32, tag="pg", bufs=3,
                                  name=f"pgw{g}"))
        for k in range(KC):
            for g in range(3):
                mm(
                    wave[g][:],
                    wps[0][:, k * 1024 + g * 128:k * 1024 + (g + 1) * 128],
                    xt_sb[:, k * BC:k * BC + BN],
                    start=(k == 0),
                    stop=(k == KC - 1),
                )
        pgs = {0: wave[0], 1: wave[1], 2: wave[2]}

        # ---- pipelined slot stream: drain slot t-3's tanh+combine first,
        # then slot t's leaf matmuls (2-slot lag hides ACT queueing jitter)
        prcs = {}
        for t, (p, bn, gl) in enumerate(slots):
            if (p, bn) not in inth:
                inth[(p, bn)] = keep.tile([128, BN], mmdt, tag=f"inth{p}{bn}",
                                          name=f"inth{p}{bn}")
            if 3 <= t <= 6:
                todo = [t - 3]
            elif 7 <= t <= 62:
                todo = [t - 3]
            elif t == 63:
                todo = [60, 61, 62]
            else:
                todo = []
            for s in todo:
                sp, sbn, sgl = slots[s]
                deferred(sp, sbn, sgl, pgs.pop(s))
            if t >= 3:
                pgs[t] = leaf_mms(p, bn, gl)

            if (p, bn, gl) == (3, 1, 2):
                # bn0 root: its last combine (D55) drained just above
                prcs[0] = psum.tile([48, BN], f32, tag="prc", bufs=2, name="prc0")
                for q in range(NPANEL):
                    root_mm(0, q, prcs[0])
                rh0 = work.tile([32, BN], mmdt, tag="rh", bufs=2, name="rh0")
                nc.scalar.activation(rh0[:], prcs[0][0:32, :], Tanh,
                                     bias=rootb_sb[:, 0:1])
                prp0 = psum.tile([1, BN], f32, tag="prp", bufs=1, name="prp0")
                mm(prp0[:], rootwp_sb[:], rh0[:], start=True, stop=True)
                ip0 = work.tile([16, BN], f32, tag="ip", bufs=2, name="ip0")
                nc.scalar.copy(ip0[:], prcs[0][32:48, :])
                rp0 = work.tile([1, BN], f32, tag="rp", bufs=2, name="rp0")
                nc.scalar.copy(rp0[:], prp0[:])
                nc.sync.dma_start(out[L:L + I, 0:BN], ip0[:])
                nc.sync.dma_start(out[L + I:NOUT, 0:BN], rp0[:])
            elif (p, bn, gl) == (3, 1, 4):
                prcs[1] = psum.tile([48, BN], f32, tag="prc", bufs=2, name="prc1")
                root_mm(1, 0, prcs[1])
            elif (p, bn, gl) in ((3, 1, 5), (3, 1, 6)):
                root_mm(1, gl - 4, prcs[1])

        # ---- tail: last combine, bn1 root, merged final store ------------
        sp, sbn, sgl = slots[63]
        deferred(sp, sbn, sgl, pgs.pop(63))
        root_mm(1, 3, prcs[1])
        rh1 = work.tile([32, BN], mmdt, tag="rh", bufs=2, name="rh1")
        nc.scalar.activation(rh1[:], prcs[1][0:32, :], Tanh, bias=rootb_sb[:, 0:1])
        prp1 = psum.tile([1, BN], f32, tag="prp", bufs=1, name="prp1")
        mm(prp1[:], rootwp_sb[:], rh1[:], start=True, stop=True)
        ip1 = work.tile([16, BN], f32, tag="ip", bufs=2, name="ip1")
        nc.scalar.copy(ip1[:], prcs[1][32:48, :])
        rp1 = work.tile([1, BN], f32, tag="rp", bufs=2, name="rp1")
        nc.scalar.copy(rp1[:], prp1[:])
        nc.sync.dma_start(out[L:L + I, BN:BC], ip1[:])
        nc.sync.dma_start(out[L + I:NOUT, BN:BC], rp1[:])

    nc.compile()
    return nc


def _pack_weights(inp):
    f = np.float32
    int_W = np.asarray(inp["int_W"], f)
    int_b = np.asarray(inp["int_b"], f)
    root_W = np.asarray(inp["root_W"], f)
    root_b = np.asarray(inp["root_b"], f)
    leaf_Wp = np.asarray(inp["leaf_Wp"], f)
    int_Wp = np.asarray(inp["int_Wp"], f)
    root_Wp = np.asarray(inp["root_Wp"], f)

    w = {}
    w["lw"] = np.ascontiguousarray(
        np.asarray(inp["leaf_W"], f).transpose(1, 0, 2).reshape(D, L * H)
    )
    w["lb"] = np.ascontiguousarray(np.asarray(inp["leaf_b"], f).reshape(32, 128).T)

    cw = np.zeros((128, I * 2 * 64), f)
    for i in range(I):
        for j in range(2):
            base = (2 * i + j) * 64
            # int_W chunk j of node i: rows (c*32+h) = child (4j+c) hidden h
            cw[:, base:base + 32] = int_W[i, 128 * j:128 * (j + 1), :]
            for c in range(4):
                lv = 8 * i + 4 * j + c
                cw[c * 32:(c + 1) * 32, base + 32 + 4 * j + c] = leaf_Wp[lv, :, 0]
    w["cw"] = cw
    w["intb"] = np.ascontiguousarray(int_b.reshape(4, 128).T)

    rw2 = np.zeros((128, NPANEL * 48), f)
    for q in range(NPANEL):
        rw2[:, 48 * q:48 * q + 32] = root_W[128 * q:128 * (q + 1), :]
        for c in range(4):
            iv = 4 * q + c
            rw2[c * 32:(c + 1) * 32, 48 * q + 32 + 4 * q + c] = int_Wp[iv, :, 0]
    w["rw2"] = rw2
    w["rootb"] = np.ascontiguousarray(root_b.reshape(32, 1))
    w["rootwp"] = np.ascontiguousarray(root_Wp.reshape(32, 1))

    import ml_dtypes

    bf16 = ml_dtypes.bfloat16
    for k in ("lw", "cw", "rw2", "rootwp"):
        w[k] = np.ascontiguousarray(w[k].astype(bf16))
    return w


def kernel(**inputs):
    import ml_dtypes

    from concourse.bass_utils import run_bass_kernel_spmd

    nc = _CACHE.get("nc")
    if nc is None:
        nc = _CACHE["nc"] = _build_nc()

    x = np.asarray(inputs["x"], np.float32)
    w = _pack_weights(inputs)
    in_maps = []
    for c in range(NCORES):
        m = dict(w)
        m["xt"] = np.ascontiguousarray(
            x[c * BC:(c + 1) * BC, :].T.astype(ml_dtypes.bfloat16)
        )
        in_maps.append(m)

    res = run_bass_kernel_spmd(nc, in_maps, core_ids=list(range(NCORES)))
    _CACHE["last_res"] = res
    outs = [np.asarray(res.results[c]["out"], np.float32) for c in range(NCORES)]
    raw = np.concatenate(outs, axis=1)  # [145, B] raw predict dots
    bias = np.concatenate([
        np.asarray(inputs["leaf_bp"], np.float32)[:, 0],
        np.asarray(inputs["int_bp"], np.float32)[:, 0],
        np.asarray(inputs["root_bp"], np.float32),
    ])
    full = np.tanh(raw + bias[:, None])[:, :, None]
    return full.astype(np.float32)


# revision 13
# speedup vs baseline: 1.0360x; 1.0251x over previous
"""Trainium2 Bass kernel for nn_CombineNode_7395933684091 (gnn_message_passing).

Hierarchy: 128 leaf terms (each D=1024 -> H=32), 16 internal terms
(concat of 8 children hiddens, 256 -> 32), 1 root (concat of 16
internal hiddens, 512 -> 32); every term also has a 1-dim predict head.
All matmuls followed by tanh.

Strategy: data-parallel over batch across 8 cores (Bc = 1024 rows per
core), weights replicated. On-chip layout keeps hidden features on the
PARTITION axis ("h^T layout": tiles are [features, batch]), so every
level's contraction is a natural PE matmul and the child-concat is just
stacking partition tiles. All matmul operands are bf16 (full-rate PE,
FWL halves LDWEIGHTS, half the DMA bytes of f32); PSUM stays f32.

The 64 (panel, batch-half, group) leaf tiles are emitted as a software-
pipelined slot stream: each slot emits its 8 accumulating leaf matmuls,
then the *previous* slot's tanh + combine matmul, so the PE never
head-of-line blocks on the ACT engine. Per-term predict heads ride as
block-diagonal columns in the combine/root stationaries; their raw dots
are copied off PSUM by the (otherwise idle) DVE and DMA'd out
unactivated — bias + tanh for the 145 predict rows happen on the host.

DMA: inputs split across the sync (xt), vector (panel-0 weights),
gpsimd (panel-1..3 weights) engine queues so issue doesn't serialize;
first chunks are quartered so the first matmul can start right after
the framework preamble.
"""

import numpy as np

B, D, H = 8192, 1024, 32
L, I, CPI = 128, 16, 8
NCORES = 8
BC = B // NCORES      # 1024 batch rows per core
BN = 512              # batch tile width (one PSUM bank of f32)
NBH = BC // BN        # 2 batch halves
KC = D // 128         # 8 contraction chunks for the leaf level
NPANEL = 4            # leaf panels (8 groups of 4 leaves each)
GPP = 8               # groups per panel
NOUT = L + I + 1      # 145

MM_DT = "bfloat16"

_CACHE = {}


def _build_nc():
    from contextlib import ExitStack

    import concourse.mybir as mybir
    import concourse.tile as tile
    from concourse import bacc

    f32 = mybir.dt.float32
    Tanh = mybir.ActivationFunctionType.Tanh
    mmdt = getattr(mybir.dt, MM_DT)

    nc = bacc.Bacc("TRN2", target_bir_lowering=False, debug=False)

    xt = nc.dram_tensor("xt", [D, BC], mmdt, kind="ExternalInput")
    lw = nc.dram_tensor("lw", [D, L * H], mmdt, kind="ExternalInput")
    lb = nc.dram_tensor("lb", [128, 32], f32, kind="ExternalInput")
    # fused internal-trans + leaf-predict stationary: per (node i, chunk j)
    # a [128, 64] block: cols 0:32 int_W chunk, cols 32+4j+c leaf Wp diag
    cw = nc.dram_tensor("cw", [128, I * 2 * 64], mmdt, kind="ExternalInput")
    intb = nc.dram_tensor("intb", [128, 4], f32, kind="ExternalInput")
    # fused root-trans + int-predict stationary: per panel q a [128, 48]
    # block: cols 0:32 root_W chunk, cols 32:48 int Wp diag
    rw2 = nc.dram_tensor("rw2", [128, NPANEL * 48], mmdt, kind="ExternalInput")
    rootb = nc.dram_tensor("rootb", [32, 1], f32, kind="ExternalInput")
    rootwp = nc.dram_tensor("rootwp", [32, 1], mmdt, kind="ExternalInput")
    # raw (pre-bias, pre-tanh) predict dots; host applies bias + tanh
    out = nc.dram_tensor("out", [NOUT, BC], f32, kind="ExternalOutput")

    mm = nc.tensor.matmul

    with tile.TileContext(nc) as tc, ExitStack() as ctx:
        consts = ctx.enter_context(tc.tile_pool(name="consts", bufs=1))
        wpool = ctx.enter_context(tc.tile_pool(name="wpool", bufs=1))
        work = ctx.enter_context(tc.tile_pool(name="work", bufs=4))
        keep = ctx.enter_context(tc.tile_pool(name="keep", bufs=1))
        psum = ctx.enter_context(tc.tile_pool(name="psum", bufs=1, space="PSUM"))

        # ---- PE pre-warm: input DMA cannot issue before the framework
        # preamble ends (~7us), so ~4us of dummy f32 matmuls bridges the
        # first-chunk transfer latency and trips the HAM clock un-throttle
        # right as real data lands.
        warm_l = nc.const_aps.tensor(0.0, (128, 48), f32)
        warm_r = nc.const_aps.tensor(0.0, (128, 64), f32)
        pwarm = psum.tile([128, BN], f32, tag="pg", bufs=4, name="pwarm")
        for _ in range(10):
            mm(pwarm[0:48, 0:64], warm_l, warm_r, start=True, stop=True,
               skip_group_check=True)
        # dummy reader so the BIR verifier keeps the warm-up writes
        wjunk = work.tile([1, 8], f32, tag="wjunk", bufs=1, name="wjunk")
        nc.scalar.copy(wjunk[:], pwarm[0:1, 0:8])

        # ---- input DMAs, split across engine queues ------------------------
        # sync: xt left halves (bn0, early chunks quartered), right halves,
        # then panel 2-3 weights
        xt_sb = consts.tile([128, KC * BC], mmdt, name="xt_sb")
        for k in range(KC):
            nc.sync.dma_start(
                xt_sb[:, k * BC:k * BC + BN], xt[k * 128:(k + 1) * 128, 0:BN]
            )

        # gpsimd: panel-0 weights (early chunks quartered), then panel 1
        wps = {}
        wps[0] = wpool.tile([128, KC * 1024], mmdt, tag="wp0", name="wp0")
        for k in range(KC):
            nc.gpsimd.dma_start(
                wps[0][:, k * 1024:k * 1024 + 512], lw[k * 128:(k + 1) * 128, 0:512]
            )
        for k in range(KC):
            nc.gpsimd.dma_start(
                wps[0][:, k * 1024 + 512:(k + 1) * 1024],
                lw[k * 128:(k + 1) * 128, 512:1024],
            )

        # scalar: small constants (then it only runs ACTs)
        lb_sb = consts.tile([128, 32], f32, name="lb_sb")
        nc.scalar.dma_start(lb_sb[:], lb[:])
        intb_sb = consts.tile([128, 4], f32, name="intb_sb")
        nc.scalar.dma_start(intb_sb[:], intb[:])
        cw_sb = consts.tile([128, I * 2 * 64], mmdt, name="cw_sb")
        nc.scalar.dma_start(cw_sb[:], cw[:])
        rw2_sb = consts.tile([128, NPANEL * 48], mmdt, name="rw2_sb")
        nc.scalar.dma_start(rw2_sb[:], rw2[:])
        rootb_sb = consts.tile([32, 1], f32, name="rootb_sb")
        nc.scalar.dma_start(rootb_sb[:], rootb[:])
        rootwp_sb = consts.tile([32, 1], mmdt, name="rootwp_sb")
        nc.scalar.dma_start(rootwp_sb[:], rootwp[:])

        # panel 1 on gpsimd (after wp0); panels 2-3 then xt right halves
        # (needed only for the bn1 pass, ~70us in) on sync
        for q, eng in ((1, nc.gpsimd), (2, nc.sync), (3, nc.sync)):
            wps[q] = wpool.tile([128, KC * 1024], mmdt, tag=f"wp{q}", name=f"wp{q}")
            for k in range(KC):
                eng.dma_start(
                    wps[q][:, k * 1024:(k + 1) * 1024],
                    lw[k * 128:(k + 1) * 128, q * 1024:(q + 1) * 1024],
                )
        for k in range(KC):
            nc.sync.dma_start(
                xt_sb[:, k * BC + BN:(k + 1) * BC],
                xt[k * 128:(k + 1) * 128, BN:BC],
            )

        inth = {}   # (panel, bn) -> [128, BN] tile: int nodes 4p..4p+3 h^T
        pcombs = {}  # il -> in-flight combine PSUM tile

        # bn-outer: full bn0 pass over all panels, then bn1 — halves the
        # byte footprint the head DMA must deliver before the PE can stream
        slots = [(p, bn, gl) for bn in range(NBH) for p in range(NPANEL)
                 for gl in range(GPP)]

        def leaf_mms(p, bn, gl):
            wp = wps[p]
            pg = psum.tile([128, BN], f32, tag="pg", bufs=4, name=f"pg{p}{bn}{gl}")
            for k in range(KC):
                mm(
                    pg[:],
                    wp[:, k * 1024 + gl * 128:k * 1024 + (gl + 1) * 128],
                    xt_sb[:, k * BC + bn * BN:k * BC + bn * BN + BN],
                    start=(k == 0),
                    stop=(k == KC - 1),
                )
            return pg

        lhs = {}

        def leaf_tanh(t):
            p, bn, gl = slots[t]
            lh = work.tile([128, BN], mmdt, tag="lh", bufs=6, name=f"lh{p}{bn}{gl}")
            nc.scalar.activation(
                lh[:], pgs.pop(t)[:], Tanh,
                bias=lb_sb[:, GPP * p + gl:GPP * p + gl + 1]
            )
            lhs[t] = lh

        def comb_pair(t0):
            """Both combine matmuls of one internal node, back to back, then
            its tanh; all PSUM readers live on the scalar queue so the ring
            reuse collapses to a single semaphore (no PE pipeline breaks)."""
            p, bn, gl = slots[t0]
            il = gl // 2
            i = 4 * p + il
            pc = psum.tile([64, BN], f32, tag="pcomb", bufs=2,
                           name=f"pc{p}{bn}{il}")
            for j, t in ((0, t0), (1, t0 + 1)):
                mm(
                    pc[:],
                    cw_sb[:, (2 * i + j) * 64:(2 * i + j + 1) * 64],
                    lhs.pop(t)[:],
                    start=(j == 0),
                    stop=(j == 1),
                    skip_group_check=True,
                )
            nc.scalar.activation(
                inth[(p, bn)][32 * il:32 * il + 32, :],
                pc[0:32, :],
                Tanh,
                bias=intb_sb[32 * il:32 * il + 32, p:p + 1],
            )
            last = (p, bn) == (NPANEL - 1, 1)
            lp = work.tile([8, BN], f32, tag="lp", bufs=8, name=f"lp{i}{bn}")
            if last and il == 3:
                nc.vector.tensor_copy(lp[:], pc[32:40, :])
            else:
                nc.scalar.copy(lp[:], pc[32:40, :])
            eng = nc.sync if (bn == 1 or p >= 2) else nc.gpsimd
            eng.dma_start(out[8 * i:8 * i + 8, bn * BN:bn * BN + BN], lp[:])

        def root_mm(bn, q, prc):
            mm(
                prc[:],
                rw2_sb[:, 48 * q:48 * (q + 1)],
                inth[(q, bn)][:],
                start=(q == 0),
                stop=(q == NPANEL - 1),
                skip_group_check=True,
            )

        # ---- head: k-outer wave over groups 0-2 of (panel 0, bn 0) so the
        # PE consumes each arriving (xt, wp0) chunk as soon as it lands
        inth[(0, 0)] = keep.tile([128, BN], mmdt, tag="inth00", name="inth00")
        wave = []
        for g in range(3):
            wave.append(psum.tile([128, BN], f32, tag="pg", bufs=4,
                                  name=f"pgw{g}"))
        for k in range(KC):
            for g in range(3):
                mm(
                    wave[g][:],
                    wps[0][:, k * 1024 + g * 128:k * 1024 + (g + 1) * 128],
                    xt_sb[:, k * BC:k * BC + BN],
                    start=(k == 0),
                    stop=(k == KC - 1),
                )
        pgs = {0: wave[0], 1: wave[1], 2: wave[2]}

        # ---- pipelined slot stream --------------------------------------
        # slot t: tanh of slot t-2, leaf matmuls of t, and at even t the
        # combine pair for slots (t-4, t-3)
        prcs = {}
        for t, (p, bn, gl) in enumerate(slots):
            if (p, bn) not in inth:
                inth[(p, bn)] = keep.tile([128, BN], mmdt, tag=f"inth{p}{bn}",
                                          name=f"inth{p}{bn}")
            if t >= 2:
                leaf_tanh(t - 2)
            if t == 63:
                leaf_tanh(62)
            if t >= 3:
                pgs[t] = leaf_mms(p, bn, gl)
            if t >= 4 and t % 2 == 0:
                comb_pair(t - 4)
            if t == 63:
                comb_pair(60)

            if t == 34:
                # bn0 root: (3,0)'s last combine just drained above
                prcs[0] = psum.tile([48, BN], f32, tag="prc", bufs=1, name="prc0")
                for q in range(NPANEL):
                    root_mm(0, q, prcs[0])
                rh0 = work.tile([32, BN], mmdt, tag="rh", bufs=2, name="rh0")
                nc.scalar.activation(rh0[:], prcs[0][0:32, :], Tanh,
                                     bias=rootb_sb[:, 0:1])
                prp0 = psum.tile([1, BN], f32, tag="prp", bufs=1, name="prp0")
                mm(prp0[:], rootwp_sb[:], rh0[:], start=True, stop=True)
                ip0 = work.tile([16, BN], f32, tag="ip", bufs=2, name="ip0")
                nc.scalar.copy(ip0[:], prcs[0][32:48, :])
                rp0 = work.tile([1, BN], f32, tag="rp", bufs=2, name="rp0")
                nc.scalar.copy(rp0[:], prp0[:])
                nc.sync.dma_start(out[L:L + I, 0:BN], ip0[:])
                nc.sync.dma_start(out[L + I:NOUT, 0:BN], rp0[:])
            elif t == 60:
                prcs[1] = psum.tile([48, BN], f32, tag="prc", bufs=1, name="prc1")
                root_mm(1, 0, prcs[1])
            elif t in (61, 62):
                root_mm(1, t - 60, prcs[1])

        # ---- tail: last tanh + combine pair, bn1 root --------------------
        leaf_tanh(63)
        comb_pair(62)
        root_mm(1, 3, prcs[1])
        rh1 = work.tile([32, BN], mmdt, tag="rh", bufs=2, name="rh1")
        nc.scalar.activation(rh1[:], prcs[1][0:32, :], Tanh, bias=rootb_sb[:, 0:1])
        prp1 = psum.tile([1, BN], f32, tag="prp", bufs=1, name="prp1")
        mm(prp1[:], rootwp_sb[:], rh1[:], start=True, stop=True)
        ip1 = work.tile([16, BN], f32, tag="ip", bufs=2, name="ip1")
        nc.vector.tensor_copy(ip1[:], prcs[1][32:48, :])
        rp1 = work.tile([1, BN], f32, tag="rp", bufs=2, name="rp1")
        nc.vector.tensor_copy(rp1[:], prp1[:])
        nc.sync.dma_start(out[L:L + I, BN:BC], ip1[:])
        nc.sync.dma_start(out[L + I:NOUT, BN:BC], rp1[:])

    nc.compile()
    return nc


def _pack_weights(inp):
    f = np.float32
    int_W = np.asarray(inp["int_W"], f)
    int_b = np.asarray(inp["int_b"], f)
    root_W = np.asarray(inp["root_W"], f)
    root_b = np.asarray(inp["root_b"], f)
    leaf_Wp = np.asarray(inp["leaf_Wp"], f)
    int_Wp = np.asarray(inp["int_Wp"], f)
    root_Wp = np.asarray(inp["root_Wp"], f)

    w = {}
    w["lw"] = np.ascontiguousarray(
        np.asarray(inp["leaf_W"], f).transpose(1, 0, 2).reshape(D, L * H)
    )
    w["lb"] = np.ascontiguousarray(np.asarray(inp["leaf_b"], f).reshape(32, 128).T)

    cw = np.zeros((128, I * 2 * 64), f)
    for i in range(I):
        for j in range(2):
            base = (2 * i + j) * 64
            # int_W chunk j of node i: rows (c*32+h) = child (4j+c) hidden h
            cw[:, base:base + 32] = int_W[i, 128 * j:128 * (j + 1), :]
            for c in range(4):
                lv = 8 * i + 4 * j + c
                cw[c * 32:(c + 1) * 32, base + 32 + 4 * j + c] = leaf_Wp[lv, :, 0]
    w["cw"] = cw
    w["intb"] = np.ascontiguousarray(int_b.reshape(4, 128).T)

    rw2 = np.zeros((128, NPANEL * 48), f)
    for q in range(NPANEL):
        rw2[:, 48 * q:48 * q + 32] = root_W[128 * q:128 * (q + 1), :]
        for c in range(4):
            iv = 4 * q + c
            rw2[c * 32:(c + 1) * 32, 48 * q + 32 + 4 * q + c] = int_Wp[iv, :, 0]
    w["rw2"] = rw2
    w["rootb"] = np.ascontiguousarray(root_b.reshape(32, 1))
    w["rootwp"] = np.ascontiguousarray(root_Wp.reshape(32, 1))

    import ml_dtypes

    bf16 = ml_dtypes.bfloat16
    for k in ("lw", "cw", "rw2", "rootwp"):
        w[k] = np.ascontiguousarray(w[k].astype(bf16))
    return w


def kernel(**inputs):
    import ml_dtypes

    from concourse.bass_utils import run_bass_kernel_spmd

    nc = _CACHE.get("nc")
    if nc is None:
        nc = _CACHE["nc"] = _build_nc()

    x = np.asarray(inputs["x"], np.float32)
    w = _pack_weights(inputs)
    in_maps = []
    for c in range(NCORES):
        m = dict(w)
        m["xt"] = np.ascontiguousarray(
            x[c * BC:(c + 1) * BC, :].T.astype(ml_dtypes.bfloat16)
        )
        in_maps.append(m)

    res = run_bass_kernel_spmd(nc, in_maps, core_ids=list(range(NCORES)))
    _CACHE["last_res"] = res
    outs = [np.asarray(res.results[c]["out"], np.float32) for c in range(NCORES)]
    raw = np.concatenate(outs, axis=1)  # [145, B] raw predict dots
    bias = np.concatenate([
        np.asarray(inputs["leaf_bp"], np.float32)[:, 0],
        np.asarray(inputs["int_bp"], np.float32)[:, 0],
        np.asarray(inputs["root_bp"], np.float32),
    ])
    full = np.tanh(raw + bias[:, None])[:, :, None]
    return full.astype(np.float32)
